# revision 1
# baseline (speedup 1.0000x reference)
"""ComplexBatchNorm2D (per-channel 2x2 covariance whitening + affine) on 8 trn2 cores.

Sharding: by channel (C=256 -> 32 channels per core). Per-channel statistics are
fully local to one core, so no collectives are needed. Each core processes its
32 channels in 8 groups of 4; a group is a [128, 4096] f32 tile pair with
partition p = (c_local*32 + b) and free = H*W. Data stays SBUF-resident between
the stats pass and the whitening apply, so HBM traffic is one read + one write.
"""

import sys

sys.path.insert(0, "/opt/trn_rl_repo")

import numpy as np

B, C, H, W = 32, 256, 64, 64
N_CORES = 8
C_PER_CORE = C // N_CORES  # 32
GROUPS = 8  # per core
C_PER_GROUP = C_PER_CORE // GROUPS  # 4
HW = H * W  # 4096
N = B * HW  # elements per channel
EPS = 1e-5

_CACHE = {}
LAST_RESULTS = None  # BassKernelResults from the most recent run (for test.py)
TRACE = False  # set True from test.py to collect an NTFF profile


def _build():
    import concourse.mybir as mybir
    import concourse.tile as tile
    from concourse.bacc import Bacc

    f32 = mybir.dt.float32
    Alu = mybir.AluOpType
    Act = mybir.ActivationFunctionType

    nc = Bacc()
    xr_d = nc.dram_tensor("xr", (B, C_PER_CORE, HW), f32, kind="ExternalInput")
    xi_d = nc.dram_tensor("xi", (B, C_PER_CORE, HW), f32, kind="ExternalInput")
    gc_d = nc.dram_tensor("gcols", (GROUPS, 128, 6), f32, kind="ExternalInput")
    out_d = nc.dram_tensor("out", (B, C_PER_CORE, 2 * HW), f32, kind="ExternalOutput")

    # Block-diagonal ones: bd[p, m] = 1 iff p//32 == m//32. One fp32 matmul with
    # this both reduces each channel's 32 b-partitions and broadcasts the result
    # back to all 128 partitions.
    bd = np.zeros((128, 128), np.float32)
    for blk in range(C_PER_GROUP):
        bd[blk * 32 : (blk + 1) * 32, blk * 32 : (blk + 1) * 32] = 1.0
    bd_d = nc.inline_tensor(bd, "bdiag")

    with tile.TileContext(nc) as tc:
        with (
            tc.tile_pool(name="io", bufs=3) as io_pool,
            tc.tile_pool(name="ob", bufs=2) as ob_pool,
            # bufs=8 = one slot per group: small tiles are never reused, so
            # no slot-release waits ever land on the ops that write them
            tc.tile_pool(name="small", bufs=8) as small_pool,
            tc.tile_pool(name="singles", bufs=1) as singles,
            tc.tile_pool(name="ps", bufs=8, space="PSUM") as ps_pool,
        ):
            bd_t = singles.tile([128, 128], f32)
            dma_bd = nc.sync.dma_start(out=bd_t, in_=bd_d[:, :])
            gc_t = singles.tile([128, GROUPS, 6], f32)
            dma_gc = nc.sync.dma_start(
                out=gc_t, in_=gc_d[:, :, :].rearrange("g p s -> p g s")
            )

            for g in range(GROUPS):
                h = {}
                cs = g * C_PER_GROUP
                xr = io_pool.tile([128, HW], f32, tag="xr")
                xi = io_pool.tile([128, HW], f32, tag="xi")
                # SBUF side must stay a flat [128, F] AP (a split partition
                # dim mis-lowers); the DRAM side carries the 3D reorder.
                # Loads and stats run per hw-half so stats start at half-load.
                HH = HW // 2
                for hh in range(2):
                    f0 = hh * HH
                    nc.sync.dma_start(
                        out=xr[:, f0 : f0 + HH],
                        in_=xr_d[:, cs : cs + C_PER_GROUP, f0 : f0 + HH]
                        .rearrange("b c f -> c b f"),
                    )
                    nc.sync.dma_start(
                        out=xi[:, f0 : f0 + HH],
                        in_=xi_d[:, cs : cs + C_PER_GROUP, f0 : f0 + HH]
                        .rearrange("b c f -> c b f"),
                    )

                ob = ob_pool.tile([128, 2 * HW], f32, tag="ob")
                ob3 = ob[:, :].rearrange("p (f two) -> p f two", two=2)
                # dump targets for value-discarded elementwise results
                scr_a = ob[:, 0:HW]
                scr_v = ob[:, HW : 2 * HW]

                # --- stats: raw sums per partition, per hw-half --------------
                # one stats tile per engine (each has a single writer engine);
                # cols hold per-half partials, combined by PSUM-accumulating
                # matmuls below
                st_a = small_pool.tile([128, 4], f32, tag="st_a")  # ACT
                st_v = small_pool.tile([128, 6], f32, tag="st_v")  # DVE
                for hh in range(2):
                    f0, ca, cv = hh * HH, 2 * hh, 3 * hh
                    xr_h, xi_h = xr[:, f0 : f0 + HH], xi[:, f0 : f0 + HH]
                    scr_ah, scr_vh = scr_a[:, f0 : f0 + HH], scr_v[:, f0 : f0 + HH]
                    nc.scalar.activation(
                        scr_ah, xr_h, Act.Square, accum_out=st_a[:, ca : ca + 1]
                    )
                    nc.scalar.activation(
                        scr_ah, xi_h, Act.Square, accum_out=st_a[:, ca + 1 : ca + 2]
                    )
                    # sum_ri: product and free-axis sum fused in one DVE op
                    nc.vector.scalar_tensor_tensor(
                        scr_vh, xr_h, 1.0, xi_h, Alu.mult, Alu.mult,
                        accum_out=st_v[:, cv + 2 : cv + 3],
                    )
                    # plain sums at 2x rate on DVE
                    nc.vector.tensor_scalar(
                        scr_vh, xr_h, 1.0, 0.0, Alu.mult, Alu.add,
                        accum_out=st_v[:, cv : cv + 1],
                    )
                    nc.vector.tensor_scalar(
                        scr_vh, xi_h, 1.0, 0.0, Alu.mult, Alu.add,
                        accum_out=st_v[:, cv + 1 : cv + 2],
                    )

                # --- aggregate over b and broadcast back (block-diag matmul) --
                # ps cols: 0 sum_r, 1 sum_i, 2 sum_ri, 3 sum_rr, 4 sum_ii;
                # the second matmul of each pair accumulates the other half
                ps = ps_pool.tile([128, 5], f32, tag="ps")
                nc.tensor.matmul(ps[:, 3:5], bd_t, st_a[:, 0:2],
                                 start=True, stop=False)
                nc.tensor.matmul(ps[:, 3:5], bd_t, st_a[:, 2:4],
                                 start=False, stop=True)
                nc.tensor.matmul(ps[:, 0:3], bd_t, st_v[:, 0:3],
                                 start=True, stop=False)
                nc.tensor.matmul(ps[:, 0:3], bd_t, st_v[:, 3:6],
                                 start=False, stop=True)

                # T columns: 0 m_r, 1 m_i, 2 e_ri, 3 e_rr, 4 e_ii, 5 a, 6 d,
                # 7 nb, 8 s0, 9 1/s0, 10 det/s0, 11 s, 12 ad, 13 nb2, 14 det,
                # 15 tr2s, 16 t0, 17 1/t0, 18 tr2s/t0, 19 t, 20 dn, 21 rdn,
                # 22 dps, 23 aps, 24:26 gb, 26:28 ga, 28:30 A00|A10,
                # 30:32 A01|A11, 32:34 t6, 34:36 t7, 36:38 bias_r|bias_i
                # T cols 0..4 = m_r, m_i, e_ri, e_rr, e_ii (ps order matches)
                T = small_pool.tile([128, 38], f32, tag="T")
                h["evac3"] = nc.scalar.activation(
                    T[:, 0:5], ps[:, 0:5], Act.Copy, scale=1.0 / N
                )

                gc = gc_t[:, g, :]
                stt = nc.vector.scalar_tensor_tensor
                tt = nc.vector.tensor_tensor
                ts = nc.vector.tensor_scalar

                # a, d = E[x^2] - m^2 + 2*EPS  (reference adds EPS to cov twice)
                stt(T[:, 5:7], T[:, 0:2], -1.0, T[:, 0:2], Alu.mult, Alu.mult)
                stt(T[:, 5:7], T[:, 5:7], 2.0 * EPS, T[:, 3:5], Alu.add, Alu.add)
                # nb = -b = m_r*m_i - E[ri]
                stt(T[:, 7:8], T[:, 0:1], T[:, 1:2], T[:, 2:3], Alu.mult, Alu.subtract)
                # det = a*d - b^2
                tt(T[:, 12:13], T[:, 5:6], T[:, 6:7], Alu.mult)
                tt(T[:, 13:14], T[:, 7:8], T[:, 7:8], Alu.mult)
                tt(T[:, 14:15], T[:, 12:13], T[:, 13:14], Alu.subtract)
                # s = sqrt(det), Newton-refined (ACT sqrt LUT is low-precision)
                nc.scalar.activation(T[:, 8:9], T[:, 14:15], Act.Sqrt)
                nc.vector.reciprocal(T[:, 9:10], T[:, 8:9])
                tt(T[:, 10:11], T[:, 14:15], T[:, 9:10], Alu.mult)
                tt(T[:, 11:12], T[:, 8:9], T[:, 10:11], Alu.add)
                ts(T[:, 11:12], T[:, 11:12], 0.5, None, Alu.mult)
                # dps = d+s, aps = a+s, tr2s = a+d+2s
                tt(T[:, 22:23], T[:, 6:7], T[:, 11:12], Alu.add)
                tt(T[:, 23:24], T[:, 5:6], T[:, 11:12], Alu.add)
                tt(T[:, 15:16], T[:, 22:23], T[:, 23:24], Alu.add)
                # t = sqrt(tr2s), Newton-refined
                nc.scalar.activation(T[:, 16:17], T[:, 15:16], Act.Sqrt)
                nc.vector.reciprocal(T[:, 17:18], T[:, 16:17])
                tt(T[:, 18:19], T[:, 15:16], T[:, 17:18], Alu.mult)
                tt(T[:, 19:20], T[:, 16:17], T[:, 18:19], Alu.add)
                ts(T[:, 19:20], T[:, 19:20], 0.5, None, Alu.mult)
                # rdn = 1/(s*t)
                tt(T[:, 20:21], T[:, 11:12], T[:, 19:20], Alu.mult)
                nc.vector.reciprocal(T[:, 21:22], T[:, 20:21])
                # A = gamma @ W, W = [[dps, nb], [nb, aps]] * rdn
                # [A00, A10] = ([g00,g10]*dps + [g01,g11]*nb) * rdn
                ts(T[:, 24:26], gc[:, 2:4], T[:, 7:8], None, Alu.mult)
                stt(T[:, 28:30], gc[:, 0:2], T[:, 22:23], T[:, 24:26], Alu.mult, Alu.add)
                ts(T[:, 28:30], T[:, 28:30], T[:, 21:22], None, Alu.mult)
                # [A01, A11] = ([g00,g10]*nb + [g01,g11]*aps) * rdn
                ts(T[:, 26:28], gc[:, 2:4], T[:, 23:24], None, Alu.mult)
                stt(T[:, 30:32], gc[:, 0:2], T[:, 7:8], T[:, 26:28], Alu.mult, Alu.add)
                ts(T[:, 30:32], T[:, 30:32], T[:, 21:22], None, Alu.mult)
                # bias' = beta - [A00,A10]*m_r - [A01,A11]*m_i
                ts(T[:, 32:34], T[:, 28:30], T[:, 0:1], None, Alu.mult)
                stt(T[:, 34:36], T[:, 30:32], T[:, 1:2], T[:, 32:34], Alu.mult, Alu.add)
                tt(T[:, 36:38], gc[:, 4:6], T[:, 34:36], Alu.subtract)

                # --- apply: out_r = A00*xr + A01*xi + br'; interleave r/i -----
                # u_r = A00*xr + br -> ob second half (read leads write in the
                # later strided STT, so the overlap is safe); u_i = A10*xr + bi
                # -> in place over xr (xr's last use). Keeps ACT decoupled
                # from the DVE ob writes with zero extra SBUF.
                u_r = scr_v
                nc.scalar.activation(
                    u_r, xr, Act.Identity, bias=T[:, 36:37], scale=T[:, 28:29]
                )
                h["a3"] = nc.scalar.activation(
                    xr, xr, Act.Identity, bias=T[:, 37:38], scale=T[:, 29:30]
                )
                # DVE apply runs per hw-half so the first half's 2 MiB store
                # can launch while the second half still computes
                HH = HW // 2
                for hh in range(2):
                    f0 = hh * HH
                    stt(
                        ob3[:, f0 : f0 + HH, 0], xi[:, f0 : f0 + HH],
                        T[:, 30:31], u_r[:, f0 : f0 + HH], Alu.mult, Alu.add,
                    )
                    stt(
                        ob3[:, f0 : f0 + HH, 1], xi[:, f0 : f0 + HH],
                        T[:, 31:32], xr[:, f0 : f0 + HH], Alu.mult, Alu.add,
                    )
                    nc.sync.dma_start(
                        out=out_d[:, cs : cs + C_PER_GROUP, 2 * f0 : 2 * f0 + HW]
                        .rearrange("b c f -> c b f"),
                        in_=ob[:, 2 * f0 : 2 * f0 + HW],
                    )
    nc.finalize()
    return nc


def kernel(x_real, x_imag, gamma, beta):
    global LAST_RESULTS
    from concourse.bass_utils import run_bass_kernel_spmd

    if "nc" not in _CACHE:
        _CACHE["nc"] = _build()
    nc = _CACHE["nc"]

    x_real = np.asarray(x_real, dtype=np.float32)
    x_imag = np.asarray(x_imag, dtype=np.float32)
    gamma = np.asarray(gamma, dtype=np.float32)
    beta = np.asarray(beta, dtype=np.float32)

    # per-channel columns [g00, g10, g01, g11, beta_r, beta_i]
    gcols_all = np.stack(
        [gamma[:, 0, 0], gamma[:, 1, 0], gamma[:, 0, 1], gamma[:, 1, 1],
         beta[:, 0], beta[:, 1]],
        axis=-1,
    ).astype(np.float32)  # (C, 6)

    in_maps = []
    for k in range(N_CORES):
        sl = slice(k * C_PER_CORE, (k + 1) * C_PER_CORE)
        gk = gcols_all[sl].reshape(GROUPS, C_PER_GROUP, 1, 6)
        gk = np.broadcast_to(gk, (GROUPS, C_PER_GROUP, 32, 6)).reshape(GROUPS, 128, 6)
        in_maps.append(
            {
                "xr": np.ascontiguousarray(x_real[:, sl].reshape(B, C_PER_CORE, HW)),
                "xi": np.ascontiguousarray(x_imag[:, sl].reshape(B, C_PER_CORE, HW)),
                "gcols": np.ascontiguousarray(gk),
            }
        )

    res = run_bass_kernel_spmd(
        nc, in_maps, core_ids=list(range(N_CORES)), trace=TRACE
    )
    LAST_RESULTS = res

    out = np.empty((B, C, H, W, 2), dtype=np.float32)
    for k in range(N_CORES):
        sl = slice(k * C_PER_CORE, (k + 1) * C_PER_CORE)
        out[:, sl] = res.results[k]["out"].reshape(B, C_PER_CORE, H, W, 2)
    return out



# revision 3
# speedup vs baseline: 1.7461x; 1.7461x over previous
"""ComplexBatchNorm2D (per-channel 2x2 covariance whitening + affine) on 8 trn2 cores.

Sharding: by channel (C=256 -> 32 channels per core); per-channel statistics are
local to one core, so no collectives. Each core processes its 32 channels in
8 groups of 4; a group is a [128, 4096] tile pair (partition p = c_local*32 + b,
free = H*W). I/O is f16 (inputs converted on host, outputs upcast on host),
halving HBM traffic vs f32; the 2e-2 rel-err budget dwarfs f16 rounding.

Engine split per group (cost-model ns):
  DVE : sums of xr, xi and of the Pool-produced xr*xi product via 4x
        tensor_scalar-accum (3x1127), whitening+affine apply as
        TS2/TS/TT-in-place (8896), plus the small per-channel chain
  ACT : Square-accum x2 (2x3785), psum evac, 2 sqrts
  Pool: prod = xr*xi via tensor_tensor (8222; only TT/TS-imm lower to Pool)
  PE  : tiny block-diag matmuls aggregating the 32 b-partitions per channel
  DMA : 4 x 1MB f16 transfers/group = 11.65us at the 360 GB/s model
Issue order is software-pipelined: stats(g), prod-sum(g-1), chain(g-1),
apply(g-2) so DVE never waits on the per-group scalar chain or on Pool.
"""

import sys

sys.path.insert(0, "/opt/trn_rl_repo")

import numpy as np

B, C, H, W = 32, 256, 64, 64
N_CORES = 8
C_PER_CORE = C // N_CORES  # 32
GROUPS = 8  # per core
C_PER_GROUP = C_PER_CORE // GROUPS  # 4
HW = H * W  # 4096
N = B * HW  # elements per channel
EPS = 1e-5

_CACHE = {}
LAST_RESULTS = None  # BassKernelResults from the most recent run (for test.py)
TRACE = False


def _build():
    import concourse.mybir as mybir
    import concourse.tile as tile
    from concourse.bacc import Bacc

    f32 = mybir.dt.float32
    f16 = mybir.dt.float16
    Alu = mybir.AluOpType
    Act = mybir.ActivationFunctionType

    nc = Bacc()
    xr_d = nc.dram_tensor("xr", (B, C_PER_CORE, HW), f16, kind="ExternalInput")
    xi_d = nc.dram_tensor("xi", (B, C_PER_CORE, HW), f16, kind="ExternalInput")
    gc_d = nc.dram_tensor("gcols", (GROUPS, 128, 6), f32, kind="ExternalInput")
    or_d = nc.dram_tensor("outr", (B, C_PER_CORE, HW), f16, kind="ExternalOutput")
    oi_d = nc.dram_tensor("outi", (B, C_PER_CORE, HW), f16, kind="ExternalOutput")

    # Block-diagonal ones: bd[p, m] = 1 iff p//32 == m//32. One matmul with this
    # both reduces each channel's 32 b-partitions and broadcasts back to 128.
    bd = np.zeros((128, 128), np.float32)
    for blk in range(C_PER_GROUP):
        bd[blk * 32 : (blk + 1) * 32, blk * 32 : (blk + 1) * 32] = 1.0
    bd_d = nc.inline_tensor(bd, "bdiag")

    with tile.TileContext(nc) as tc:
        with (
            tc.tile_pool(name="io", bufs=3) as io_pool,
            tc.tile_pool(name="pl", bufs=3) as pl_pool,
            tc.tile_pool(name="u", bufs=2) as u_pool,
            tc.tile_pool(name="pr", bufs=2) as pr_pool,
            tc.tile_pool(name="small", bufs=8) as small_pool,
            tc.tile_pool(name="singles", bufs=1) as singles,
            tc.tile_pool(name="ps", bufs=8, space="PSUM") as ps_pool,
        ):
            bd_t = singles.tile([128, 128], f32)
            nc.sync.dma_start(out=bd_t, in_=bd_d[:, :])
            gc_t = singles.tile([128, GROUPS, 6], f32)
            nc.sync.dma_start(
                out=gc_t, in_=gc_d[:, :, :].rearrange("g p s -> p g s")
            )
            # value-discarded dump targets, one per writer engine
            scr_a = singles.tile([128, HW], f16)
            scr_v = singles.tile([128, HW], f16)

            st = {}  # group -> (st_a, st_v, nh)
            Ts = {}  # group -> T tile
            xs = {}  # group -> (xr, xi)
            prods = {}  # group -> scr_p tile
            pss = {}  # group -> psum tile
            stt = nc.vector.scalar_tensor_tensor
            tt = nc.vector.tensor_tensor
            ts = nc.vector.tensor_scalar

            def nhalves(g):
                # last group streams in halves so its stats finish right after
                # the final input byte lands (shorter drain)
                return 2 if g == GROUPS - 1 else 1

            def stage_load_stats(g):
                nh = nhalves(g)
                cs = g * C_PER_GROUP
                xr = io_pool.tile([128, HW], f16, tag="xr")
                xi = io_pool.tile([128, HW], f16, tag="xi")
                scr_p = pr_pool.tile([128, HW], f16, tag="scr_p")
                st_a = small_pool.tile([128, nh, 2], f32, tag="st_a")
                st_v = small_pool.tile([128, nh, 3], f32, tag="st_v")
                FH = HW // nh
                for h in range(nh):
                    sl = slice(h * FH, (h + 1) * FH)
                    nc.sync.dma_start(
                        out=xr[:, sl],
                        in_=xr_d[:, cs : cs + C_PER_GROUP, sl]
                        .rearrange("b c f -> c b f"),
                    )
                    nc.sync.dma_start(
                        out=xi[:, sl],
                        in_=xi_d[:, cs : cs + C_PER_GROUP, sl]
                        .rearrange("b c f -> c b f"),
                    )
                for h in range(nh):
                    sl = slice(h * FH, (h + 1) * FH)
                    nc.scalar.activation(
                        scr_a[:, sl], xr[:, sl], Act.Square,
                        accum_out=st_a[:, h, 0:1],
                    )
                    nc.scalar.activation(
                        scr_a[:, sl], xi[:, sl], Act.Square,
                        accum_out=st_a[:, h, 1:2],
                    )
                    ts(scr_v[:, sl], xr[:, sl], 1.0, 0.0, Alu.mult, Alu.add,
                       accum_out=st_v[:, h, 0:1])
                    ts(scr_v[:, sl], xi[:, sl], 1.0, 0.0, Alu.mult, Alu.add,
                       accum_out=st_v[:, h, 1:2])
                    nc.gpsimd.tensor_tensor(
                        scr_p[:, sl], xr[:, sl], xi[:, sl], Alu.mult
                    )
                st[g] = (st_a, st_v, nh)
                xs[g] = (xr, xi)
                prods[g] = scr_p
                # aggregate what's ready now (sums + squares)
                ps = ps_pool.tile([128, 5], f32, tag="ps")
                pss[g] = ps
                for h in range(nh):
                    nc.tensor.matmul(ps[:, 0:2], bd_t, st_v[:, h, 0:2],
                                     start=(h == 0), stop=(h == nh - 1))
                for h in range(nh):
                    nc.tensor.matmul(ps[:, 3:5], bd_t, st_a[:, h, 0:2],
                                     start=(h == 0), stop=(h == nh - 1))

            def stage_prodsum(g):
                # sum the Pool-made product (4x TS-accum) and aggregate it
                st_a, st_v, nh = st[g]
                scr_p = prods.pop(g)
                FH = HW // nh
                for h in range(nh):
                    sl = slice(h * FH, (h + 1) * FH)
                    ts(scr_v[:, sl], scr_p[:, sl], 1.0, 0.0, Alu.mult, Alu.add,
                       accum_out=st_v[:, h, 2:3])
                ps = pss[g]
                for h in range(nh):
                    nc.tensor.matmul(ps[:, 2:3], bd_t, st_v[:, h, 2:3],
                                     start=(h == 0), stop=(h == nh - 1))

            def stage_chainA(g):
                # T cols: 0 m_r, 1 m_i, 2 e_ri, 3 e_rr, 4 e_ii, 5 a, 6 d,
                # 7 nb, 8 ad, 9 nb2, 10 det, 11 apd, 12 s, 13 tr2s, 14 t,
                # 15 st, 16 rdn, 17 dps, 18 aps, 19:21 gnb, 21:23 uA00|uA10,
                # 23:25 gaps, 25:27 uA01|uA11, 27:29 A00|A10, 29:31 A01|A11,
                # 31:33 Am_r, 33:35 Am, 35:37 bias_r|bias_i
                T = small_pool.tile([128, 37], f32, tag="T")
                Ts[g] = T
                gc = gc_t[:, g, :]
                nc.scalar.activation(T[:, 0:5], pss.pop(g)[:, 0:5], Act.Copy,
                                     scale=1.0 / N)
                stt(T[:, 5:7], T[:, 0:2], -1.0, T[:, 0:2], Alu.mult, Alu.mult)
                stt(T[:, 5:7], T[:, 5:7], 2.0 * EPS, T[:, 3:5], Alu.add, Alu.add)
                stt(T[:, 7:8], T[:, 0:1], T[:, 1:2], T[:, 2:3],
                    Alu.mult, Alu.subtract)
                tt(T[:, 8:9], T[:, 5:6], T[:, 6:7], Alu.mult)
                tt(T[:, 9:10], T[:, 7:8], T[:, 7:8], Alu.mult)
                tt(T[:, 10:11], T[:, 8:9], T[:, 9:10], Alu.subtract)
                tt(T[:, 11:12], T[:, 5:6], T[:, 6:7], Alu.add)
                nc.scalar.activation(T[:, 12:13], T[:, 10:11], Act.Sqrt)
                stt(T[:, 13:14], T[:, 12:13], 2.0, T[:, 11:12], Alu.mult, Alu.add)
                nc.scalar.activation(T[:, 14:15], T[:, 13:14], Act.Sqrt)
                tt(T[:, 15:16], T[:, 12:13], T[:, 14:15], Alu.mult)
                tt(T[:, 17:18], T[:, 6:7], T[:, 12:13], Alu.add)
                tt(T[:, 18:19], T[:, 5:6], T[:, 12:13], Alu.add)
                ts(T[:, 19:21], gc[:, 2:4], T[:, 7:8], None, Alu.mult)
                stt(T[:, 21:23], gc[:, 0:2], T[:, 17:18], T[:, 19:21],
                    Alu.mult, Alu.add)
                ts(T[:, 23:25], gc[:, 2:4], T[:, 18:19], None, Alu.mult)
                stt(T[:, 25:27], gc[:, 0:2], T[:, 7:8], T[:, 23:25],
                    Alu.mult, Alu.add)

            def stage_recip(g):
                nc.vector.reciprocal(Ts[g][:, 16:17], Ts[g][:, 15:16])

            def stage_chainB(g):
                T = Ts[g]
                gc = gc_t[:, g, :]
                ts(T[:, 27:29], T[:, 21:23], T[:, 16:17], None, Alu.mult)
                ts(T[:, 29:31], T[:, 25:27], T[:, 16:17], None, Alu.mult)
                ts(T[:, 31:33], T[:, 27:29], T[:, 0:1], None, Alu.mult)
                stt(T[:, 33:35], T[:, 29:31], T[:, 1:2], T[:, 31:33],
                    Alu.mult, Alu.add)
                tt(T[:, 35:37], gc[:, 4:6], T[:, 33:35], Alu.subtract)

            def stage_apply_store(g):
                # out_r = A00*xr + A01*xi + br'; out_i = A10*xr + A11*xi + bi'
                T = Ts.pop(g)
                xr, xi = xs.pop(g)
                cs = g * C_PER_GROUP
                nh = nhalves(g)
                FH = HW // nh
                orp = pl_pool.tile([128, HW], f16, tag="orp")
                oip = pl_pool.tile([128, HW], f16, tag="oip")
                u = u_pool.tile([128, HW], f16, tag="u")
                for h in range(nh):
                    sl = slice(h * FH, (h + 1) * FH)
                    ts(u[:, sl], xr[:, sl], T[:, 27:28], T[:, 35:36],
                       Alu.mult, Alu.add)
                    ts(xr[:, sl], xr[:, sl], T[:, 28:29], T[:, 36:37],
                       Alu.mult, Alu.add)
                    ts(orp[:, sl], xi[:, sl], T[:, 29:30], None, Alu.mult)
                    tt(orp[:, sl], orp[:, sl], u[:, sl], Alu.add)
                    ts(oip[:, sl], xi[:, sl], T[:, 30:31], None, Alu.mult)
                    tt(oip[:, sl], oip[:, sl], xr[:, sl], Alu.add)
                    nc.sync.dma_start(
                        out=or_d[:, cs : cs + C_PER_GROUP, sl]
                        .rearrange("b c f -> c b f"),
                        in_=orp[:, sl],
                    )
                    nc.sync.dma_start(
                        out=oi_d[:, cs : cs + C_PER_GROUP, sl]
                        .rearrange("b c f -> c b f"),
                        in_=oip[:, sl],
                    )

            for it in range(GROUPS + 2):
                if it < GROUPS:
                    stage_load_stats(it)
                k = it - 2
                if 0 <= k < GROUPS:
                    stage_apply_store(k)
                j = it - 1
                if 0 <= j < GROUPS:
                    stage_prodsum(j)
                    stage_chainA(j)
                    stage_recip(j)
                    stage_chainB(j)
    nc.finalize()
    return nc


def kernel(x_real, x_imag, gamma, beta):
    global LAST_RESULTS
    from concourse.bass_utils import run_bass_kernel_spmd

    if "nc" not in _CACHE:
        _CACHE["nc"] = _build()
    nc = _CACHE["nc"]

    xr16 = np.asarray(x_real, dtype=np.float16).reshape(B, C, HW)
    xi16 = np.asarray(x_imag, dtype=np.float16).reshape(B, C, HW)
    gamma = np.asarray(gamma, dtype=np.float32)
    beta = np.asarray(beta, dtype=np.float32)

    # per-channel columns [g00, g10, g01, g11, beta_r, beta_i]
    gcols_all = np.stack(
        [gamma[:, 0, 0], gamma[:, 1, 0], gamma[:, 0, 1], gamma[:, 1, 1],
         beta[:, 0], beta[:, 1]],
        axis=-1,
    ).astype(np.float32)  # (C, 6)

    in_maps = []
    for k in range(N_CORES):
        sl = slice(k * C_PER_CORE, (k + 1) * C_PER_CORE)
        gk = gcols_all[sl].reshape(GROUPS, C_PER_GROUP, 1, 6)
        gk = np.broadcast_to(gk, (GROUPS, C_PER_GROUP, 32, 6)).reshape(GROUPS, 128, 6)
        in_maps.append(
            {
                "xr": np.ascontiguousarray(xr16[:, sl]),
                "xi": np.ascontiguousarray(xi16[:, sl]),
                "gcols": np.ascontiguousarray(gk),
            }
        )

    res = run_bass_kernel_spmd(
        nc, in_maps, core_ids=list(range(N_CORES)), trace=TRACE
    )
    LAST_RESULTS = res

    out = np.empty((B, C, H, W, 2), dtype=np.float32)
    for k in range(N_CORES):
        sl = slice(k * C_PER_CORE, (k + 1) * C_PER_CORE)
        out[:, sl, :, :, 0] = res.results[k]["outr"].reshape(B, C_PER_CORE, H, W)
        out[:, sl, :, :, 1] = res.results[k]["outi"].reshape(B, C_PER_CORE, H, W)
    return out
